# revision 45
# baseline (speedup 1.0000x reference)
"""Multi-head self-attention (B=4, T=2048, E=1024, H=16, Dh=64) on 8 trn2 cores.

Sharding (tensor-parallel over heads + data-parallel over batch, per the
problem's sharding hint): core c handles batch b=c//2 and head-half s=c%2
(8 of 16 heads), with ALL 2048 queries of its batch. Each core computes
q/k/v projections for its 512 head-dims, attention for its 8 heads, and a
PARTIAL output projection (contracting only its 512 rows of fc_w). The two
partials per batch are summed on the host (the TP all-reduce), plus fc_b.

All matmul operands are fp16 (fp32 PSUM accumulation): fp16 streams 1
col/cycle through the PE vs 0.5 for fp32. Softmax denominators come free
from a ones-column in the packed V block (baseline trick); reciprocals are
taken on the two denominator rows BEFORE the mask-matmul broadcast.
exp() runs as [128,2048] ACTIVATEs (4 PSUM banks) to amortize the ~350cyc
fixed cost. Phase A (projections) of group g+1 is software-pipelined into
phase B (attention) of group g, and the fc matmuls of query-window w are
pipelined into phase B iterations of window w+1, so the in-order PE queue
always has fill work during exp stalls.

Per-core math (heads processed in NG=2 groups of 4; head pair p packs its
even head at partitions 0:64 and odd head at 64:128):
  xt        [E, T]            (input, host pre-transposed/tiled, fp16)
  qT_j      [64, 2048] = Wq_j.T @ xt + bq_j      (slab-packed, zero-padded)
  kT_j      [64, 2048] = Wk_j.T @ xt             (k-bias is softmax-invariant)
  v_j       [2048, 64] = xt.T @ Wv_j + bv_j      (+ ones column for denom)
  e         [k,q] blocks via K=128 matmul over the zero-padded slabs
  p = exp(e/8);  psO = [v | ones].T @ p  -> AV rows + denominator row
  outT      [64, q] per head = AV rows * recip(denom) (broadcast via dmask)
  out_part  [2048, 1024] = outT.T @ fc_w[s*512:(s+1)*512, :]   (partial)
"""

import numpy as np

E = 1024
T = 2048          # tokens per batch (= queries = keys per core)
H = 16
DH = 64
EC = E // 128     # 8 e-chunks
NG = 2            # head groups per core
HPG = 4           # heads per group
GC = HPG // 2     # head pairs per group = 2
NH = NG * HPG     # heads per core = 8
HD = NH * DH      # head-dims per core = 512
N_CORES = 8
SCALE = DH ** -0.5
KC = T // 128     # 16 key chunks
NTB = T // 512    # 4 token blocks
NQB = T // 512    # 4 query windows

_CACHE = {}


def _build():
    import concourse.bass as bass
    import concourse.mybir as mybir
    import concourse.tile as tile
    from concourse import bacc
    from contextlib import ExitStack

    f32 = mybir.dt.float32
    f32r = mybir.dt.float32r
    f16 = mybir.dt.float16
    AF = mybir.ActivationFunctionType
    OP = mybir.AluOpType

    nc = bacc.Bacc("TRN2", target_bir_lowering=False, debug=False)

    xt4 = nc.declare_dram_parameter("xt4", [NTB, 128, EC * 512], f16, isOutput=False).ap()
    wqkv = nc.declare_dram_parameter("wqkv", [NG, 3, 128, EC * 256], f16, isOutput=False).ap()
    q_bias = nc.declare_dram_parameter("q_bias", [128, NG * GC, 1], f32, isOutput=False).ap()
    v_bias = nc.declare_dram_parameter("v_bias", [128, HD], f32, isOutput=False).ap()
    fc_w = nc.declare_dram_parameter("fc_w", [128, (HD // 128) * E], f16, isOutput=False).ap()
    out = nc.declare_dram_parameter("out", [T, E], f32, isOutput=True).ap()

    with tile.TileContext(nc) as tc, ExitStack() as ctx:
        pool_const = ctx.enter_context(tc.tile_pool(name="const", bufs=1))
        pool_kqv = ctx.enter_context(tc.tile_pool(name="kqv", bufs=1))
        pool_w = ctx.enter_context(tc.tile_pool(name="w", bufs=2))
        pool_xt = ctx.enter_context(tc.tile_pool(name="xt", bufs=3))
        pool_ex = ctx.enter_context(tc.tile_pool(name="ex", bufs=4))
        pool_outT = ctx.enter_context(tc.tile_pool(name="outT", bufs=1))
        pool_fc = ctx.enter_context(tc.tile_pool(name="fc", bufs=1))
        pool_ot = ctx.enter_context(tc.tile_pool(name="ot", bufs=4))
        ps_mm = ctx.enter_context(tc.tile_pool(name="psmm", bufs=2, space="PSUM"))
        ps_e = ctx.enter_context(tc.tile_pool(name="pse", bufs=1, space="PSUM"))
        ps_o = ctx.enter_context(tc.tile_pool(name="pso", bufs=1, space="PSUM"))

        # ---- constants / one-time inits (overlap the first weight DMAs) ----
        vb_sb = pool_const.tile([128, HD], f32)
        qb_sb = pool_const.tile([128, NG * GC, 1], f32)
        fcw_sb = pool_fc.tile([128, HD // 128, E], f16)

        # denominator-broadcast mask: psR = dmask.T @ dsb maps dsb row 64
        # (even-head denom) onto rows 0:64 and dsb row 0 (odd) onto 64:128
        dmask = pool_const.tile([128, 128], f16)
        nc.vector.memset(dmask, 0.0)
        nc.vector.memset(dmask[64:65, 0:64], 1.0)
        nc.vector.memset(dmask[0:1, 64:128], 1.0)

        # warm the exp table set off the critical path (no DMA dependency)
        dummy = pool_const.tile([128, 1], f16)
        nc.scalar.activation(dummy, dmask[:, 0:1], AF.Exp, scale=0.125)

        dsbs = []
        for i in range(2):
            d = pool_const.tile([128, 512], f16, name=f"dsb{i}")
            nc.vector.memset(d, 0.0)
            dsbs.append(d)

        # double-buffered k/q/v group tiles; pad regions zeroed once per buffer
        # pads on gpsimd: keeps the vector queue free for phase-A copies
        kTs, qTs, vas = [], [], []
        for i in range(2):
            kT = pool_kqv.tile([128, HPG, T], f16, name=f"kT{i}", tag=f"kT{i}")
            qT = pool_kqv.tile([128, HPG, T], f16, name=f"qT{i}", tag=f"qT{i}")
            va = pool_kqv.tile([128, KC, GC, 192], f16, name=f"va{i}", tag=f"va{i}")
            for j in range(HPG):
                zlo, zhi = (64, 128) if j % 2 == 0 else (0, 64)
                nc.gpsimd.memset(kT[zlo:zhi, j, :], 0.0)
                nc.gpsimd.memset(qT[zlo:zhi, j, :], 0.0)
            nc.gpsimd.memset(va[:, :, :, 64:65], 1.0)
            nc.gpsimd.memset(va[:, :, :, 65:128], 0.0)
            kTs.append(kT)
            qTs.append(qT)
            vas.append(va)

        outT = pool_outT.tile([128, HD // 128, T], f16)

        # ---- phase A: projections for group g, as a list of chunk closures ----
        def a_chunks(g):
            gi = g % 2
            kT, qT, va = kTs[gi], qTs[gi], vas[gi]
            wq = pool_w.tile([128, EC, 256], f16, name=f"wq{g}", tag="wq")
            wk = pool_w.tile([128, EC, 256], f16, name=f"wk{g}", tag="wk")
            wv = pool_w.tile([128, EC, 256], f16, name=f"wv{g}", tag="wv")

            def dma_w():
                # DMA in chunk consumption order: k-proj, v-proj, q-proj
                for w, m in ((wk, 1), (wv, 2), (wq, 0)):
                    nc.sync.dma_start(
                        out=w, in_=wqkv[g, m].rearrange("p (c n) -> p c n", c=EC))

            def dma_first():
                # startup path: interleave wk and xt halves on the serial DMA
                # queue so the first accumulation chain starts mid-transfer
                wk_src = wqkv[g, 1].rearrange("p (c n) -> p c n", c=EC)
                xt_tb = pool_xt.tile([128, EC, 512], f16, name=f"xt{g}_0", tag="xt")
                src = xt4[0].rearrange("p (c n) -> p c n", c=EC)
                for h in range(4):
                    s = slice(2 * h, 2 * h + 2)
                    nc.sync.dma_start(out=wk[:, s, :], in_=wk_src[:, s, :])
                    nc.sync.dma_start(out=xt_tb[:, s, :], in_=src[:, s, :])
                for w, m in ((wv, 2), (wq, 0)):
                    nc.sync.dma_start(
                        out=w, in_=wqkv[g, m].rearrange("p (c n) -> p c n", c=EC))
                xt_tbs[0] = xt_tb

            chunks = []
            xt_tbs = {}

            def mk_tb(tb):
                def dma_xt():
                    xt_tb = pool_xt.tile([128, EC, 512], f16, name=f"xt{g}_{tb}", tag="xt")
                    nc.sync.dma_start(
                        out=xt_tb, in_=xt4[tb].rearrange("p (c n) -> p c n", c=EC))
                    xt_tbs[tb] = xt_tb
                return dma_xt

            def mk_kproj(tb, ch):
                def f():
                    xt_tb = xt_tbs[tb]
                    psA = ps_mm.tile([128, 512], f32, tag="mm")
                    for ec in range(EC):
                        nc.tensor.matmul(
                            psA, wk[:, ec, ch * 128:(ch + 1) * 128], xt_tb[:, ec, :],
                            start=(ec == 0), stop=(ec == EC - 1))
                    nc.vector.tensor_copy(
                        kT[0:64, 2 * ch, tb * 512:(tb + 1) * 512], psA[0:64, :])
                    nc.vector.tensor_copy(
                        kT[64:128, 2 * ch + 1, tb * 512:(tb + 1) * 512], psA[64:128, :])
                return f

            def mk_qproj(tb, ch):
                def f():
                    xt_tb = xt_tbs[tb]
                    P = g * GC + ch
                    psQ = ps_mm.tile([128, 512], f32, tag="mm")
                    for ec in range(EC):
                        nc.tensor.matmul(
                            psQ, wq[:, ec, ch * 128:(ch + 1) * 128], xt_tb[:, ec, :],
                            start=(ec == 0), stop=(ec == EC - 1))
                    nc.vector.tensor_scalar(
                        qT[0:64, 2 * ch, tb * 512:(tb + 1) * 512],
                        psQ[0:64, :], qb_sb[0:64, P, :], None, OP.add)
                    nc.vector.tensor_scalar(
                        qT[64:128, 2 * ch + 1, tb * 512:(tb + 1) * 512],
                        psQ[64:128, :], qb_sb[64:128, P, :], None, OP.add)
                return f

            def mk_vproj(tb, tc_):
                def f():
                    xt_tb = xt_tbs[tb]
                    psV = ps_mm.tile([128, 256], f32, tag="mm")
                    for ec in range(EC):
                        nc.tensor.matmul(
                            psV, xt_tb[:, ec, tc_ * 128:(tc_ + 1) * 128], wv[:, ec, :],
                            start=(ec == 0), stop=(ec == EC - 1))
                    psv4 = psV.rearrange("p (pr h d) -> p pr h d", pr=GC, h=2)
                    vb4 = vb_sb[:, g * 256:(g + 1) * 256].rearrange(
                        "p (pr h d) -> p pr h d", pr=GC, h=2)
                    K0 = tb * 4 + tc_
                    nc.vector.tensor_add(
                        va[:, K0, :, 0:64], psv4[:, :, 0, :], vb4[:, :, 0, :])
                    nc.vector.tensor_add(
                        va[:, K0, :, 128:192], psv4[:, :, 1, :], vb4[:, :, 1, :])
                return f

            for tb in range(NTB):
                if tb == 0:
                    pre = [dma_first] if g == 0 else [dma_w, mk_tb(0)]
                else:
                    pre = []
                first = mk_kproj(tb, 0)
                chunks.append((pre, first))
                # v before q: when a tb's chunks are deferred into B(g+1),
                # the AV consumers of va appear earlier than any qT consumer
                # prefetch the next token block's xt right after this block
                # starts (bufs=3 keeps three tiles in flight)
                pre2 = [mk_tb(tb + 1)] if tb + 1 < NTB else []
                chunks.append((pre2, mk_kproj(tb, 1)))
                for f in (mk_vproj(tb, 0), mk_vproj(tb, 1),
                          mk_vproj(tb, 2), mk_vproj(tb, 3), mk_qproj(tb, 0),
                          mk_qproj(tb, 1)):
                    chunks.append(([], f))
            return chunks

        def run_chunk(c):
            pre, f = c
            for p in pre:
                p()
            f()

        # ---- phase C: one fc unit = one 128x512 output block ----
        def fc_unit(qc, ob):
            psC = ps_mm.tile([128, 512], f32, tag="mm")
            for hc in range(HD // 128):
                nc.tensor.matmul(
                    psC, outT[:, hc, qc * 128:(qc + 1) * 128],
                    fcw_sb[:, hc, ob * 512:(ob + 1) * 512],
                    start=(hc == 0), stop=(hc == HD // 128 - 1))
            ot = pool_ot.tile([128, 512], f32, tag="ot")
            nc.vector.tensor_copy(ot, psC)
            nc.sync.dma_start(
                out=out[qc * 128:(qc + 1) * 128, ob * 512:(ob + 1) * 512], in_=ot)

        # ---- phase B: attention for (g, qb-window, head pair p) ----
        def b_iter(g, qb, p, feeder, it, feed_ks=(3, 7, 11, 15)):
            gi = g % 2
            kT, qT, va = kTs[gi], qTs[gi], vas[gi]
            psO_e = ps_o.tile([128, 512], f32, tag="po_e")
            psO_o = ps_o.tile([128, 512], f32, tag="po_o")

            def av(K, ex):
                blk = va[:, K, p, :]
                # even head: 65 stationary cols (64 dims + ones) suffice —
                # psO_e rows 65:127 are never read
                nc.tensor.matmul(
                    psO_e[0:65, :], blk[:, 0:65], ex[:, 0, :],
                    start=(K == 0), stop=(K == KC - 1))
                nc.tensor.matmul(
                    psO_o, blk[:, 64:192], ex[:, 1, :],
                    start=(K == 0), stop=(K == KC - 1))

            # software pipeline: AV of chunk K-1 is emitted after QK/ACT of
            # chunk K, so the in-order PE queue never stalls on ACT(K)
            prev = None
            for K in range(KC):
                psE = ps_e.tile([128, 2, 512], f32, tag=f"pe{K % 2}")
                for hf in range(2):
                    j = p * 2 + hf
                    # contract over just the 64 data rows of the slab
                    # (K=64): halves the stationary load vs the padded K=128
                    lo = 0 if j % 2 == 0 else 64
                    nc.tensor.matmul(
                        psE[:, hf, :],
                        kT[lo:lo + 64, j, K * 128:(K + 1) * 128],
                        qT[lo:lo + 64, j, qb * 512:(qb + 1) * 512],
                        start=True, stop=True)
                ex = pool_ex.tile([128, 2, 512], f16, tag="ex")
                nc.scalar.activation(ex, psE, AF.Exp, scale=SCALE)
                if prev is not None:
                    av(*prev)
                if K in feed_ks:
                    feeder()  # fill remaining slack with proj/fc matmuls
                prev = (K, ex)
            av(*prev)
            dsb = dsbs[it % 2]
            nc.vector.tensor_copy(dsb[64:65, :], psO_e[64:65, :])
            nc.vector.tensor_copy(dsb[0:1, :], psO_o[0:1, :])
            psR = ps_mm.tile([128, 512], f32, tag="mm")
            # rows 1..63 of dmask/dsb are all-zero: K=65 covers both
            # nonzero rows (0 and 64) exactly
            nc.tensor.matmul(psR, dmask[0:65, :], dsb[0:65, :],
                             start=True, stop=True)
            recipb = pool_ex.tile([128, 512], f32, tag="recipb")
            nc.vector.reciprocal_approx_fast(out=recipb, in_=psR)
            P = g * GC + p
            qs = slice(qb * 512, (qb + 1) * 512)
            nc.vector.tensor_tensor(
                outT[0:64, P, qs], psO_e[0:64, :], recipb[0:64, :], OP.mult)
            nc.vector.tensor_tensor(
                outT[64:128, P, qs], psO_o[64:128, :], recipb[64:128, :], OP.mult)

        # ---- main schedule ----
        ch0 = a_chunks(0)
        run_chunk(ch0[0])
        # biases ride behind the first weight/xt DMAs (needed ~10us in)
        nc.sync.dma_start(out=vb_sb, in_=v_bias)
        nc.sync.dma_start(out=qb_sb, in_=q_bias)
        for c in ch0[1:]:
            run_chunk(c)
        # fc weights are only needed in B(1); DMA after group-0 weights/xt
        nc.sync.dma_start(out=fcw_sb, in_=fc_w.rearrange("p (c n) -> p c n", c=HD // 128))

        # one shared fill queue: A(1) chunks first, fc units appended as
        # their query windows complete. B(0) drains 24, B(1) the rest.
        feed_items = list(a_chunks(1))
        state = {"i": 0}

        def feeder():
            if state["i"] < len(feed_items):
                run_chunk(feed_items[state["i"]])
                state["i"] += 1

        it = 0
        for qb in range(NQB):
            for p in range(GC):
                b_iter(0, qb, p, feeder, it, feed_ks=(2, 6, 10, 14))
                it += 1
        while state["i"] < len(feed_items):
            feeder()
        for qb in range(NQB):
            for p in range(GC):
                b_iter(1, qb, p, feeder, it, feed_ks=(2, 5, 8, 11, 14))
                it += 1
            for qc in range(qb * 4, (qb + 1) * 4):
                for ob in range(2):
                    feed_items.append(([], (lambda qc=qc, ob=ob: fc_unit(qc, ob))))
        while state["i"] < len(feed_items):
            feeder()

    nc.compile()
    return nc


def _get_nc():
    if "nc" not in _CACHE:
        _CACHE["nc"] = _build()
    return _CACHE["nc"]


def _in_maps(x, qkv_w, qkv_b, fc_w, fc_b):
    f16 = np.float16
    x = np.asarray(x, dtype=np.float32)
    qkv_w = np.asarray(qkv_w, dtype=np.float32)
    qkv_b = np.asarray(qkv_b, dtype=np.float32)
    fc_w = np.asarray(fc_w, dtype=np.float32)

    maps = []
    for c in range(N_CORES):
        b, s = c // 2, c % 2
        # xt4[tb, p, ec*512 + t] = x[b, tb*512 + t, ec*128 + p]
        xt4 = np.ascontiguousarray(
            x[b].astype(f16).reshape(NTB, 512, EC, 128).transpose(0, 3, 2, 1)
            .reshape(NTB, 128, EC * 512))
        # wqkv[g, m, p, ec*256 + n] = W_m[ec*128 + p, s*512 + g*256 + n]
        wg = np.empty((NG, 3, 128, EC * 256), dtype=f16)
        for m in range(3):
            sub = qkv_w[:, m * E + s * HD:m * E + (s + 1) * HD]  # [E, 512]
            for g in range(NG):
                blk = sub[:, g * 256:(g + 1) * 256]              # [E, 256]
                wg[g, m] = (blk.reshape(EC, 128, 256).transpose(1, 0, 2)
                            .reshape(128, EC * 256).astype(f16))
        qb = np.ascontiguousarray(
            qkv_b[s * HD:(s + 1) * HD].reshape(NG * GC, 128).T.reshape(128, NG * GC, 1),
            dtype=np.float32)
        vb = np.ascontiguousarray(
            np.broadcast_to(qkv_b[2 * E + s * HD:2 * E + (s + 1) * HD], (128, HD)),
            dtype=np.float32)
        # fc_ws[p, hc*1024 + n] = fc_w[s*512 + hc*128 + p, n]
        fcs = np.ascontiguousarray(
            fc_w[s * HD:(s + 1) * HD, :].reshape(HD // 128, 128, E)
            .transpose(1, 0, 2).reshape(128, (HD // 128) * E).astype(f16))
        maps.append({"xt4": xt4, "wqkv": wg, "q_bias": qb, "v_bias": vb,
                     "fc_w": fcs})
    return maps


def run(x, qkv_w, qkv_b, fc_w, fc_b, trace=False):
    from concourse.bass_utils import run_bass_kernel_spmd

    nc = _get_nc()
    maps = _in_maps(x, qkv_w, qkv_b, fc_w, fc_b)
    res = run_bass_kernel_spmd(nc, maps, list(range(N_CORES)), trace=trace)
    B = np.asarray(x).shape[0]
    fc_b = np.asarray(fc_b, dtype=np.float32)
    full = np.empty((B, T, E), dtype=np.float32)
    for b in range(B):
        full[b] = res.results[2 * b]["out"] + res.results[2 * b + 1]["out"] + fc_b
    return full, res


def kernel(x, qkv_w, qkv_b, fc_w, fc_b):
    full, _ = run(x, qkv_w, qkv_b, fc_w, fc_b, trace=False)
    return full
